# revision 1
# baseline (speedup 1.0000x reference)
"""AttnBlock (GroupNorm -> 1x1 qkv conv -> full HW x HW attention -> 1x1 proj
-> residual) on 8 Trainium2 NeuronCores.

Sharding: 8 cores = 4 batch elements x 2 query-halves. Each core receives its
batch element's full x[b] (pixel axis rolled so the core's query half sits in
columns 0..2047), computes GroupNorm + full K/V + Q for its half, runs
attention over key blocks, and the output projection. The host transposes the
1x1-conv weights, adds proj bias + residual, and gathers.

Raw Bass (explicit per-engine streams + semaphores; this toolchain's walrus
rejects the multi-wait instructions Tile emits). Compute dtype bf16 for all
big matmuls (fp32 accumulation in PSUM); GroupNorm statistics in fp32.

Device layouts (partition dim first):
  h  = groupnormed x, bf16   [C=512 -> 4 tiles of 128, HW=4096]
  Q  = wqT.T @ h (+bq)       [512 -> 4 tiles, 2048]
  K  = wkT.T @ h (+bk)       [512 -> 4 tiles, 4096]
  Vt = h.T @ wvT (+bv)       [128, 32 j-blocks, 512]   (pixels on partitions)
  scores_t = K.T @ Q         [128 keys, 512 queries] psum, per (j, i-quarter)
  probs    = exp(scores * C^-0.5), bf16   (no max subtraction; |scores| < ~6)
  O       += Vt_j.T @ probs_j   [4 x [128, 512]] psum accumulated over j
  sums    += ones.T @ probs_j   [1, 512] psum
  out = (wpT.T @ O) * (1/sums broadcast)  -> DRAM [512, 2048] f32
"""

from contextlib import ExitStack

import numpy as np

import concourse.bass as bass
from concourse import mybir
from concourse.bass_utils import run_bass_kernel_spmd

F32 = mybir.dt.float32
BF16 = mybir.dt.bfloat16

B, C, H, W = 4, 512, 64, 64
HW = H * W              # 4096 pixels
NG = 32                 # groupnorm groups
GS = C // NG            # 16 channels per group
P = 128                 # SBUF partitions
KC = C // P             # 4 channel chunks
NQ = HW // 2            # 2048 queries per core
F = 512                 # free-dim tile (one PSUM bank of f32)
NJ = HW // P            # 32 key blocks
NQF = NQ // F           # 4 query quarters
NGT = P // GS           # 8 groups per channel tile
EPS = 1e-6
SCALE = float(C) ** -0.5
AF = mybir.ActivationFunctionType
ALU = mybir.AluOpType


def build_nc() -> bass.Bass:
    nc = bass.Bass()

    x_d = nc.dram_tensor("x", [C, HW], F32, kind="ExternalInput")
    w_d = {}
    for nm in ("wqT", "wkT", "wvT", "wpT"):
        w_d[nm] = nc.dram_tensor(nm, [C, C], F32, kind="ExternalInput")
    bq_d = nc.dram_tensor("bq", [C, 1], F32, kind="ExternalInput")
    bk_d = nc.dram_tensor("bk", [C, 1], F32, kind="ExternalInput")
    bvb_d = nc.dram_tensor("bvb", [P, C], F32, kind="ExternalInput")
    gsc_d = nc.dram_tensor("gscale", [C, 1], F32, kind="ExternalInput")
    gbi_d = nc.dram_tensor("gbias", [C, 1], F32, kind="ExternalInput")
    gmat_d = nc.dram_tensor("gmat", [P, NGT], F32, kind="ExternalInput")
    gexp_d = nc.dram_tensor("gexp", [NGT, P], F32, kind="ExternalInput")
    out_d = nc.dram_tensor("out", [C, NQ], F32, kind="ExternalOutput")

    ctx = ExitStack()
    with ctx:
        # ---------------- SBUF ----------------
        def sb(shape, dt, name):
            return ctx.enter_context(nc.sbuf_tensor(name, shape, dt))
        x_sb = [sb([P, HW], F32, f"x{k}") for k in range(2)]        # 32KB/p
        h_sb = [sb([P, HW], BF16, f"h{k}") for k in range(KC)]      # 32KB/p
        q_sb = [sb([P, NQ], BF16, f"q{k}") for k in range(KC)]      # 16KB/p
        k_sb = [sb([P, HW], BF16, f"kk{k}") for k in range(KC)]     # 32KB/p
        vt_sb = sb([P, NJ, F], BF16, "vt")                          # 32KB/p
        wstage = [sb([P, C], F32, f"wstage{i}") for i in range(2)]  # 4KB/p
        w_sb = {nm: [sb([P, C], BF16, f"{nm}{k}") for k in range(KC)]
                for nm in ("wqT", "wkT", "wvT", "wpT")}             # 16KB/p
        bvb_sb = sb([P, C], F32, "bvb_sb")
        gmat_sb = sb([P, NGT], F32, "gmat_sb")
        gexp_sb = sb([NGT, P], F32, "gexp_sb")
        bq_sb = [sb([P, 1], F32, f"bq_sb{k}") for k in range(KC)]
        bk_sb = [sb([P, 1], F32, f"bk_sb{k}") for k in range(KC)]
        gsc_sb = [sb([P, 1], F32, f"gsc_sb{k}") for k in range(KC)]
        gbi_sb = [sb([P, 1], F32, f"gbi_sb{k}") for k in range(KC)]
        ones_col = sb([P, 1], BF16, "ones_col")
        ones_row = sb([1, P], F32, "ones_row")
        zero_col = sb([P, 1], F32, "zero_col")
        # groupnorm scratch (per c-tile, reused)
        stats = sb([P, HW // F, 6], F32, "stats")
        mv = sb([P, 2], F32, "mv")
        st2 = sb([P, 2], F32, "st2")
        g2 = sb([NGT, 2], F32, "g2")
        gv = sb([NGT, 1], F32, "gv")
        eps_sb = sb([NGT, 1], F32, "eps_sb")
        chs = sb([P, 2], F32, "chs")
        av = sb([P, 1], F32, "av")
        bv_ = sb([P, 1], F32, "bv_")
        # attention scratch
        probs = [sb([P, F], BF16, f"probs{i}") for i in range(2)]
        recip = sb([1, F], F32, "recip")
        rb_sb = sb([P, F], F32, "rb_sb")
        o_sb = [sb([P, F], BF16, f"o_sb{i}") for i in range(KC)]
        ot = [sb([P, F], F32, f"ot{i}") for i in range(2)]

        # ---------------- PSUM (8 banks) ----------------
        def ps(shape, name):
            return ctx.enter_context(nc.psum_tensor(name, shape, F32))
        s_ps = [ps([P, F], f"s_ps{i}") for i in range(2)]
        o_ps = [ps([P, F], f"o_ps{i}") for i in range(KC)]
        sums_ps = ps([1, F], "sums_ps")
        aux_ps = ps([P, F], "aux_ps")   # gn pg/pb + recip broadcast

        # ---------------- semaphores ----------------
        def sem(name):
            return ctx.enter_context(nc.semaphore(name))
        dma_x = [sem("dma_x0"), sem("dma_x1")]   # +16 per x tile, by slot
        dma_w = [sem("dma_w0"), sem("dma_w1")]   # +16 per wstage load
        dma_m = sem("dma_m")        # +16 per misc const load
        dma_o = [sem("dma_o0"), sem("dma_o1")]   # +16 per output store
        s_wcvt = sem("s_wcvt")      # DVE memsets (4) + weight converts (16)
        s_dve = sem("s_dve")        # serialized gn DVE chain (21 per c-tile)
        s_rb = sem("s_rb")          # rb_sb copy per quarter (DVE)
        s_gn_pe = sem("s_gn_pe")    # gn PE matmuls (2 per c-tile)
        s_gn_act = sem("s_gn_act")  # gn sqrt (1 per c-tile)
        s_h = sem("s_h")            # normalized h tiles
        s_qg_pe = sem("s_qg_pe")    # qkv matmul groups done (PE)
        s_qg_dve = sem("s_qg_dve")  # qkv drains done (DVE)
        s_sc = sem("s_sc")          # scores groups (PE)
        s_exp = sem("s_exp")        # exps (ACT)
        s_att = sem("s_att")        # attnV+sums groups (PE)
        s_recip = sem("s_recip")    # recip per quarter (DVE)
        s_bcast = sem("s_bcast")    # bcast matmul per quarter (PE)
        s_osb = sem("s_osb")        # o_sb drains (DVE)
        s_pp = sem("s_pp")          # proj matmul groups (PE)
        s_ot = sem("s_ot")          # ot muls (DVE)

        NMISC = 3 + 4 * KC          # gmat, gexp, bvb, per-k consts
        W_ORDER = ("wqT", "wkT", "wvT", "wpT")

        # qkv "groups" in PE emission order
        qkv_groups = ([("v", j) for j in range(NJ)]
                      + [("q", m, n) for m in range(KC)
                         for n in range(NQ // F)]
                      + [("k", m, n) for m in range(KC)
                         for n in range(HW // F)])
        NQG = len(qkv_groups)

        with nc.Block() as block:

            # ================= GPSIMD: all DMA =================
            @block.gpsimd
            def _(g: bass.BassEngine):
                g.dma_start(out=gmat_sb[:, :], in_=gmat_d[:, :]).then_inc(
                    dma_m, 16)
                g.dma_start(out=gexp_sb[:, :], in_=gexp_d[:, :]).then_inc(
                    dma_m, 16)
                g.dma_start(out=bvb_sb[:, :], in_=bvb_d[:, :]).then_inc(
                    dma_m, 16)
                for k in range(KC):
                    sl = slice(k * P, (k + 1) * P)
                    g.dma_start(out=bq_sb[k][:, :], in_=bq_d[sl, :]).then_inc(
                        dma_m, 16)
                    g.dma_start(out=bk_sb[k][:, :], in_=bk_d[sl, :]).then_inc(
                        dma_m, 16)
                    g.dma_start(out=gsc_sb[k][:, :],
                                in_=gsc_d[sl, :]).then_inc(dma_m, 16)
                    g.dma_start(out=gbi_sb[k][:, :],
                                in_=gbi_d[sl, :]).then_inc(dma_m, 16)
                # output stores: 4 per quarter through 2 ot buffers
                for qq in range(NQF):
                    for o4 in range(KC):
                        n_out = 4 * qq + o4 + 1
                        g.wait_ge(s_ot, n_out)
                        g.dma_start(
                            out=out_d[o4 * P:(o4 + 1) * P,
                                      qq * F:(qq + 1) * F],
                            in_=ot[n_out % 2][:, :]).then_inc(
                            dma_o[n_out % 2], 16)

            # ====== SYNC: big loads on HWDGE (parallel to gpsimd) ======
            @block.sync
            def _(s: bass.BassEngine):
                for k in range(2):
                    s.dma_start(out=x_sb[k][:, :],
                                in_=x_d[k * P:(k + 1) * P, :]).then_inc(
                        dma_x[k % 2], 16)
                for i in range(4 * KC):
                    nm, k = W_ORDER[i // KC], i % KC
                    if i >= 2:
                        s.wait_ge(s_wcvt, 4 + i - 1)
                    s.dma_start(out=wstage[i % 2][:, :],
                                in_=w_d[nm][k * P:(k + 1) * P, :]).then_inc(
                        dma_w[i % 2], 16)
                for k in range(2, KC):
                    s.wait_ge(s_h, k - 1)       # x staging slot free
                    s.dma_start(out=x_sb[k % 2][:, :],
                                in_=x_d[k * P:(k + 1) * P, :]).then_inc(
                        dma_x[k % 2], 16)

            # ================= PE: all matmuls =================
            @block.tensor
            def _(t: bass.BassEngine):
                # --- groupnorm group-combine + broadcast matmuls ---
                t.wait_ge(dma_m, 16 * NMISC)
                for k in range(KC):
                    t.wait_ge(s_dve, 21 * k + 12)       # st2 ready
                    nc.tensor.matmul(aux_ps[0:NGT, 0:2], lhsT=gmat_sb[:, :],
                                     rhs=st2[:, :], start=True,
                                     stop=True).then_inc(s_gn_pe, 1)
                    t.wait_ge(s_dve, 21 * k + 17)       # g2 (mu, rstd) ready
                    nc.tensor.matmul(aux_ps[0:P, 0:2], lhsT=gexp_sb[:, :],
                                     rhs=g2[:, :], start=True,
                                     stop=True).then_inc(s_gn_pe, 1)
                # --- qkv matmuls ---
                t.wait_ge(s_wcvt, 4 + 4 * KC)           # memsets + weights
                t.wait_ge(s_h, KC)                      # all h tiles
                for gi, grp in enumerate(qkv_groups):
                    if gi >= 2:
                        t.wait_ge(s_qg_dve, gi - 1)     # psum slot free
                    dst = s_ps[gi % 2][:, :]
                    for k in range(KC):
                        kw = dict(start=(k == 0), stop=(k == KC - 1))
                        if grp[0] == "v":
                            j = grp[1]
                            mm = nc.tensor.matmul(
                                dst, lhsT=h_sb[k][:, j * P:(j + 1) * P],
                                rhs=w_sb["wvT"][k][:, :], **kw)
                        else:
                            _, m, n = grp
                            wname = "wqT" if grp[0] == "q" else "wkT"
                            mm = nc.tensor.matmul(
                                dst,
                                lhsT=w_sb[wname][k][:, m * P:(m + 1) * P],
                                rhs=h_sb[k][:, n * F:(n + 1) * F], **kw)
                    mm.then_inc(s_qg_pe, 1)
                # --- attention + proj ---
                for qq in range(NQF):
                    qsl = slice(qq * F, (qq + 1) * F)

                    def scores(j):
                        if qq == 0 and j < 2:
                            # s_ps slots still cycling out of the qkv phase
                            t.wait_ge(s_qg_dve, NQG - 1 + j)
                        else:
                            t.wait_ge(s_exp, 32 * qq + j - 1)
                        if qq > 0 and j < 2:
                            # previous quarter's proj results still leave
                            # s_ps[j] until the ot muls read them
                            t.wait_ge(s_ot, 4 * (qq - 1) + 3 + j)
                        if j == 0 and qq > 0:
                            t.wait_ge(s_osb, 4 * qq)    # O psum slots free
                        for k in range(KC):
                            mm = nc.tensor.matmul(
                                s_ps[j % 2][:, :],
                                lhsT=k_sb[k][:, j * P:(j + 1) * P],
                                rhs=q_sb[k][:, qsl],
                                start=(k == 0), stop=(k == KC - 1))
                        mm.then_inc(s_sc, 1)

                    def attnv(j):
                        t.wait_ge(s_exp, 32 * qq + j + 1)   # probs[j] ready
                        kw = dict(start=(j == 0), stop=(j == NJ - 1))
                        nc.tensor.matmul(sums_ps[:, :], lhsT=ones_col[:, :],
                                         rhs=probs[j % 2][:, :], **kw)
                        for c4 in range(KC):
                            mm = nc.tensor.matmul(
                                o_ps[c4][:, :],
                                lhsT=vt_sb[:, j, c4 * P:(c4 + 1) * P],
                                rhs=probs[j % 2][:, :], **kw)
                        mm.then_inc(s_att, 1)

                    scores(0)
                    scores(1)
                    for j in range(2, NJ):
                        scores(j)
                        attnv(j - 2)
                    attnv(NJ - 2)
                    attnv(NJ - 1)
                    # broadcast 1/sums to 128 partitions (full fp32 matmul)
                    t.wait_ge(s_recip, qq + 1)
                    if qq > 0:
                        t.wait_ge(s_rb, qq)     # aux_ps read by prior rb copy
                    nc.tensor.matmul(aux_ps[:, :], lhsT=ones_row[:, :],
                                     rhs=recip[:, :], start=True,
                                     stop=True).then_inc(s_bcast, 1)
                    # proj
                    t.wait_ge(s_osb, 4 * (qq + 1))      # all o_sb drained
                    for o4 in range(KC):
                        if o4 >= 2:
                            # s_ps slot shared with proj group o4-2: wait for
                            # its ot mul to have read the result
                            t.wait_ge(s_ot, 4 * qq + o4 - 1)
                        for c4 in range(KC):
                            mm = nc.tensor.matmul(
                                s_ps[o4 % 2][:, :],
                                lhsT=w_sb["wpT"][c4][:, o4 * P:(o4 + 1) * P],
                                rhs=o_sb[c4][:, :],
                                start=(c4 == 0), stop=(c4 == KC - 1))
                        mm.then_inc(s_pp, 1)

            # ================= DVE =================
            @block.vector
            def _(v: bass.BassEngine):
                # memsets first (counted in s_wcvt), then weight converts
                nc.vector.memset(ones_col[:, :], 1.0).then_inc(s_wcvt, 1)
                nc.vector.memset(ones_row[:, :], 1.0).then_inc(s_wcvt, 1)
                nc.vector.memset(zero_col[:, :], 0.0).then_inc(s_wcvt, 1)
                nc.vector.memset(eps_sb[:, :], EPS).then_inc(s_wcvt, 1)
                for i in range(4 * KC):
                    nm, k = W_ORDER[i // KC], i % KC
                    v.wait_ge(dma_w[i % 2], 16 * (i // 2 + 1))
                    nc.vector.tensor_copy(
                        out=w_sb[nm][k][:, :],
                        in_=wstage[i % 2][:, :]).then_inc(s_wcvt, 1)
                v.wait_ge(dma_m, 16 * NMISC)
                # groupnorm: fully serialized DVE chain (s_dve), 21 ops/tile
                ndve = 0

                def step(op):
                    nonlocal ndve
                    op.then_inc(s_dve, 1)
                    ndve += 1

                for k in range(KC):
                    if k > 0:
                        v.wait_ge(s_h, k)       # previous tile fully done
                    v.wait_ge(dma_x[k % 2], 16 * (k // 2 + 1))
                    for c8 in range(HW // F):
                        if ndve:
                            v.wait_ge(s_dve, ndve)
                        step(nc.vector.bn_stats(
                            out=stats[:, c8, :],
                            in_=x_sb[k % 2][:, c8 * F:(c8 + 1) * F]))
                    v.wait_ge(s_dve, ndve)
                    step(nc.vector.bn_aggr(out=mv[:, :], in_=stats[:, :, :]))
                    v.wait_ge(s_dve, ndve)
                    step(nc.vector.tensor_copy(out=st2[:, 0:1],
                                               in_=mv[:, 0:1]))
                    v.wait_ge(s_dve, ndve)
                    step(nc.vector.tensor_mul(out=st2[:, 1:2], in0=mv[:, 0:1],
                                              in1=mv[:, 0:1]))
                    v.wait_ge(s_dve, ndve)
                    step(nc.vector.tensor_add(out=st2[:, 1:2],
                                              in0=st2[:, 1:2],
                                              in1=mv[:, 1:2]))   # 21k+12
                    v.wait_ge(s_gn_pe, 2 * k + 1)           # pg in aux_ps
                    v.wait_ge(s_dve, ndve)
                    step(nc.vector.tensor_scalar_mul(g2[:, :],
                                                     in0=aux_ps[0:NGT, 0:2],
                                                     scalar1=1.0 / GS))
                    v.wait_ge(s_dve, ndve)
                    step(nc.vector.tensor_mul(out=gv[:, :], in0=g2[:, 0:1],
                                              in1=g2[:, 0:1]))
                    v.wait_ge(s_dve, ndve)
                    step(nc.vector.tensor_sub(out=gv[:, :], in0=g2[:, 1:2],
                                              in1=gv[:, :]))     # 21k+15
                    v.wait_ge(s_gn_act, k + 1)              # sqrt done
                    step(nc.vector.reciprocal(out=gv[:, :], in_=gv[:, :]))
                    v.wait_ge(s_dve, ndve)
                    step(nc.vector.tensor_copy(out=g2[:, 1:2],
                                               in_=gv[:, :]))    # 21k+17
                    v.wait_ge(s_gn_pe, 2 * k + 2)           # pb in aux_ps
                    v.wait_ge(s_dve, ndve)
                    step(nc.vector.tensor_copy(out=chs[:, :],
                                               in_=aux_ps[0:P, 0:2]))
                    v.wait_ge(s_dve, ndve)
                    step(nc.vector.tensor_mul(out=av[:, :], in0=chs[:, 1:2],
                                              in1=gsc_sb[k][:, :]))
                    v.wait_ge(s_dve, ndve)
                    step(nc.vector.tensor_mul(out=bv_[:, :], in0=chs[:, 0:1],
                                              in1=av[:, :]))
                    v.wait_ge(s_dve, ndve)
                    step(nc.vector.tensor_sub(out=bv_[:, :],
                                              in0=gbi_sb[k][:, :],
                                              in1=bv_[:, :]))    # 21k+21
                    v.wait_ge(s_dve, ndve)
                    nc.vector.tensor_scalar(
                        out=h_sb[k][:, :], in0=x_sb[k % 2][:, :],
                        scalar1=av[:, :], scalar2=bv_[:, :],
                        op0=ALU.mult, op1=ALU.add).then_inc(s_h, 1)
                # qkv drains
                for gi, grp in enumerate(qkv_groups):
                    v.wait_ge(s_qg_pe, gi + 1)
                    src = s_ps[gi % 2][:, :]
                    if grp[0] == "v":
                        j = grp[1]
                        op = nc.vector.tensor_add(
                            out=vt_sb[:, j, :], in0=src, in1=bvb_sb[:, :])
                    elif grp[0] == "q":
                        _, m, n = grp
                        op = nc.vector.tensor_scalar_add(
                            out=q_sb[m][:, n * F:(n + 1) * F], in0=src,
                            scalar1=bq_sb[m][:, :])
                    else:
                        _, m, n = grp
                        op = nc.vector.tensor_scalar_add(
                            out=k_sb[m][:, n * F:(n + 1) * F], in0=src,
                            scalar1=bk_sb[m][:, :])
                    op.then_inc(s_qg_dve, 1)
                # attention epilogue per quarter
                for qq in range(NQF):
                    v.wait_ge(s_att, 32 * (qq + 1))
                    if qq > 0:
                        v.wait_ge(s_bcast, qq)  # recip read by prior bcast
                    nc.vector.reciprocal(
                        out=recip[:, :],
                        in_=sums_ps[:, :]).then_inc(s_recip, 1)
                    for c4 in range(KC):
                        if qq > 0:
                            v.wait_ge(s_pp, 4 * qq)     # o_sb read by proj
                        nc.vector.tensor_copy(
                            out=o_sb[c4][:, :],
                            in_=o_ps[c4][:, :]).then_inc(s_osb, 1)
                    v.wait_ge(s_bcast, qq + 1)
                    if qq > 0:
                        v.wait_ge(s_ot, 4 * qq)     # rb_sb read by prior ots
                    nc.vector.tensor_copy(
                        out=rb_sb[:, :], in_=aux_ps[:, :]).then_inc(s_rb, 1)
                    for o4 in range(KC):
                        n_out = 4 * qq + o4 + 1
                        v.wait_ge(s_pp, n_out)
                        v.wait_ge(s_rb, qq + 1)
                        if n_out > 2:
                            # store n_out-2 (same parity slot) complete
                            cnt = ((n_out - 1) // 2 if n_out % 2 == 1
                                   else (n_out - 2) // 2)
                            v.wait_ge(dma_o[n_out % 2], 16 * cnt)
                        nc.vector.tensor_mul(
                            out=ot[n_out % 2][:, :],
                            in0=s_ps[o4 % 2][:, :],
                            in1=rb_sb[:, :]).then_inc(s_ot, 1)

            # ================= ACT: sqrt + exp =================
            @block.scalar
            def _(a: bass.BassEngine):
                a.wait_ge(s_wcvt, 4)            # memsets (eps, zero) done
                for k in range(KC):
                    a.wait_ge(s_dve, 21 * k + 15)
                    nc.scalar.activation(
                        out=gv[:, :], in_=gv[:, :], func=AF.Sqrt,
                        bias=eps_sb[:, :]).then_inc(s_gn_act, 1)
                for qq in range(NQF):
                    for j in range(NJ):
                        a.wait_ge(s_sc, 32 * qq + j + 1)
                        if 32 * qq + j >= 2:
                            a.wait_ge(s_att, 32 * qq + j - 1)
                        nc.scalar.activation(
                            out=probs[j % 2][:, :], in_=s_ps[j % 2][:, :],
                            func=AF.Exp, bias=zero_col[:, :],
                            scale=SCALE).then_inc(s_exp, 1)

    return nc


def make_in_maps(x, gn_scale, gn_bias, qkv_w, qkv_b, proj_w, proj_b):
    xf = np.ascontiguousarray(x, dtype=np.float32).reshape(B, C, HW)
    wq, wk, wv = qkv_w[0:C], qkv_w[C:2 * C], qkv_w[2 * C:3 * C]
    shared = {
        "wqT": np.ascontiguousarray(wq.T, np.float32),
        "wkT": np.ascontiguousarray(wk.T, np.float32),
        "wvT": np.ascontiguousarray(wv.T, np.float32),
        "wpT": np.ascontiguousarray(proj_w.T, np.float32),
        "bq": np.ascontiguousarray(qkv_b[0:C].reshape(C, 1), np.float32),
        "bk": np.ascontiguousarray(qkv_b[C:2 * C].reshape(C, 1), np.float32),
        "bvb": np.ascontiguousarray(
            np.broadcast_to(qkv_b[2 * C:3 * C][None, :], (P, C)), np.float32),
        "gscale": np.ascontiguousarray(gn_scale.reshape(C, 1), np.float32),
        "gbias": np.ascontiguousarray(gn_bias.reshape(C, 1), np.float32),
        "gmat": np.ascontiguousarray(
            (np.arange(P)[:, None] // GS == np.arange(NGT)[None, :]),
            np.float32),
        "gexp": np.ascontiguousarray(
            (np.arange(NGT)[:, None] == np.arange(P)[None, :] // GS),
            np.float32),
    }
    in_maps = []
    for b in range(B):
        for half in range(2):
            xr = np.ascontiguousarray(np.roll(xf[b], -half * NQ, axis=1))
            in_maps.append({"x": xr, **shared})
    return in_maps, xf


def assemble(results, xf, proj_b):
    out = np.empty((B, C, HW), np.float32)
    i = 0
    for b in range(B):
        for half in range(2):
            out[b][:, half * NQ:(half + 1) * NQ] = results[i]["out"]
            i += 1
    out += np.asarray(proj_b, np.float32)[None, :, None]
    out += xf
    return out.reshape(B, C, H, W)


def kernel(x, gn_scale, gn_bias, qkv_w, qkv_b, proj_w, proj_b):
    in_maps, xf = make_in_maps(x, gn_scale, gn_bias, qkv_w, qkv_b,
                               proj_w, proj_b)
    nc = build_nc()
    res = run_bass_kernel_spmd(nc, in_maps, list(range(8)))
    return assemble(res.results, xf, proj_b)



# revision 41
# speedup vs baseline: 2.4481x; 2.4481x over previous
"""AttnBlock (GroupNorm -> 1x1 qkv conv -> full HW x HW attention -> 1x1 proj
-> residual) on 8 Trainium2 NeuronCores.

Sharding: 8 cores = 4 batch elements x 2 query-halves. Each core receives its
batch element's full x[b] (pixel axis rolled so the core's query half sits in
columns 0..2047), computes GroupNorm + full K/V + Q for its half, runs
attention over key blocks, and the output projection. The host pre-quantizes
the 1x1-conv weights to fp8 (e4m3) in DoubleRow pair layout, and adds
proj bias + wp@bv + residual after gathering.

All big matmuls run in fp8e4 with MatmulPerfMode.DoubleRow: each instruction
contracts 256 channels (2 fp8 values per PE cell) at 0.5 cycles/row -> 4x
bf16 throughput. Pair layout convention: channel c = ch*256 + i*128 + p maps
to tensor[p, ch, i, ...].

The attention epilogue normalizes BEFORE the output projection so the fp8
staging of the attention output stays in range:
  probs = exp(scores*SCALE - SHIFT)   (SHIFT cancels in the normalization,
                                       keeps probs < 56, under the fp8 max)
  o8    = o_ps * recip(sums)          fp8 (sums matmul vs a [P,2,128]
                                       ones block lands pre-broadcast)
  out   = wpT.T @ o8                  psum -> ot staging -> DRAM

Engine plan:
  SP   : input DMA (misc consts, x tiles, fp8 weights) + output stores
  PE   : GN group-combine matmuls, all fp8 DR matmuls, recip broadcast
  DVE  : GN stats/chain, half the qkv drains, recip, rb copy, o8 + ot
  ACT  : GN sqrt + h8 writes (a*x+b in fp8), half the drains, all exps
  (GPSIMD cannot touch PSUM on trn2, so Pool is unused)

The qkv matmuls stage through bank quads (sets 0-3/4-7 alternating, drains
alternate DVE/ACT by quad parity); attention rotates scores+proj over banks
0-2 with consumers exp (s_exp) and ot-copy (s_ot), accumulates attnV output
in banks 3-6, and keeps softmax sums in bank 7.
"""

from contextlib import ExitStack

import numpy as np
import ml_dtypes

import concourse.bass as bass
from concourse import mybir
from concourse.bass_utils import run_bass_kernel_spmd

F32 = mybir.dt.float32
BF16 = mybir.dt.bfloat16
F8 = mybir.dt.float8e4

B, C, H, W = 4, 512, 64, 64
HW = H * W              # 4096 pixels
NG = 32                 # groupnorm groups
GS = C // NG            # 16 channels per group
P = 128                 # SBUF partitions
NT = C // P             # 4 channel tiles (GN granularity)
CH = 2                  # fp8 pair chunks (256 channels each)
NQ = HW // 2            # 2048 queries per core
F = 512                 # free-dim tile (one PSUM bank of f32)
NJ = HW // P            # 32 key blocks
NPAIR = NJ // 2         # 16 key-block pairs
NQF = NQ // F           # 4 query quarters
NGT = P // GS           # 8 groups per channel tile
EPS = 1e-6
SCALE = float(C) ** -0.5
SHIFT = 2.0             # exp(s*SCALE - SHIFT): cancels in normalization
AF = mybir.ActivationFunctionType
ALU = mybir.AluOpType
DR = mybir.MatmulPerfMode.DoubleRow

W_ORDER = ("wq", "wk", "wv", "wp")

# qkv work organized as QUADS of 4 matmul groups staged in a 4-bank psum
# set (sets alternate banks 0-3 / 4-7; drains alternate DVE/ACT by quad
# parity, so per-set ordering follows one engine's program order).
# K quad (n): 4 m-groups -> one 2048-wide drain into k8[:, :, :, nF:(n+1)F]
# V quad (jq): j=4jq..4jq+3  -> one drain into vt8[:, 2jq:2jq+2, :, :]
# Q quad (n): 4 single drains (per-m bias) into q8 slices
QUADS = ([("q", 0)] + [("k", n) for n in range(4)]
         + [("v", jq) for jq in range(4)]
         + [("k", n) for n in range(4, 8)]
         + [("v", jq) for jq in range(4, 8)]
         + [("q", n) for n in range(1, 4)])
NQUAD = len(QUADS)      # 20
QUAD_ENG = [q % 2 for q in range(NQUAD)]    # 0 = DVE, 1 = ACT

# (engine, drain-instr-count-after) for each quad's full drain
QUAD_DRAIN = []
_cnt = [0, 0]
for _q, (_kind, _n) in enumerate(QUADS):
    _cnt[QUAD_ENG[_q]] += {"q": 4, "k": 1, "v": 1}[_kind]
    QUAD_DRAIN.append((QUAD_ENG[_q], _cnt[QUAD_ENG[_q]]))

Q_DONE = {}             # q8 n-block -> drain reqs
K_DONE = {}             # k8 n-block (512 keys) -> drain reqs
V_DONE = {}             # v pair-block b -> drain reqs
for _q, (_kind, _n) in enumerate(QUADS):
    if _kind == "q":
        Q_DONE[_n] = [QUAD_DRAIN[_q]]
    elif _kind == "k":
        K_DONE[_n] = [QUAD_DRAIN[_q]]
    else:
        V_DONE[2 * _n] = V_DONE[2 * _n + 1] = [QUAD_DRAIN[_q]]
# last quad using each bank (for the first attention uses of that bank)
LAST_QUAD_IN_BANK = {b: max(q for q in range(NQUAD) if (q % 2) * 4 <= b < (q % 2) * 4 + 4)
                     for b in range(8)}

# attention-phase emission order of bank-0-2 uses: per quarter, scores
# j=8..31 (j=0..7 of quarter qq>0 are emitted inside quarter qq-1's epilogue
# so ACT stays fed through the recip/o8/proj boundary chain), then the
# 8 lookahead scores, then proj.
LOOKAHEAD = 8
EMISSION = []
for _qq in range(NQF):
    _j0 = 0 if _qq == 0 else LOOKAHEAD
    for _j in range(_j0, NJ):
        EMISSION.append(("score", _qq, _j))
    if _qq + 1 < NQF:
        for _j in range(LOOKAHEAD):
            EMISSION.append(("score", _qq + 1, _j))
    for _o4 in range(NT):
        EMISSION.append(("proj", _qq, _o4))
SCORE_U = {}
PROJ_U = {}
for _u, (_kind, _a, _b) in enumerate(EMISSION):
    if _kind == "score":
        SCORE_U[(_a, _b)] = _u
    else:
        PROJ_U[(_a, _b)] = _u


def _drain_waits(lst):
    best = {}
    for e, cnt in lst:
        best[e] = max(best.get(e, 0), cnt)
    return sorted(best.items())


def build_nc() -> bass.Bass:
    nc = bass.Bass()

    x_d = nc.dram_tensor("x", [C, HW], F32, kind="ExternalInput")
    w_d = {nm: nc.dram_tensor(f"{nm}8", [P, 2, CH * C], F8,
                              kind="ExternalInput") for nm in W_ORDER}
    # columns: bq(0..3) | bk(0..3) | gscale(0..3) | gbias(0..3), per c-tile
    misc_d = nc.dram_tensor("misc16", [P, 16], F32, kind="ExternalInput")
    gmat_d = nc.dram_tensor("gmat", [P, NGT], F32, kind="ExternalInput")
    gexp_d = nc.dram_tensor("gexp", [NGT, P], F32, kind="ExternalInput")
    out_d = nc.dram_tensor("out", [C, NQ], F32, kind="ExternalOutput")

    ctx = ExitStack()
    with ctx:
        # ---------------- SBUF ----------------
        def sb(shape, dt, name):
            return ctx.enter_context(nc.sbuf_tensor(name, shape, dt))
        x_sb = [sb([P, HW], F32, f"x{k}") for k in range(NT)]       # 64KB/p
        # all fp8 DoubleRow operands are [P, 2(pair), free] — walrus's dual
        # fp8 lowering requires the pair dim at AP dim 1, so chunk (ch) gets
        # its own tensor rather than a 4-dim slice
        h8 = [sb([P, 2, HW], F8, f"h8_{ch}") for ch in range(CH)]   # 16KB/p
        q8 = [sb([P, 2, NQ], F8, f"q8_{ch}") for ch in range(CH)]   # 8KB/p
        k8 = sb([P, 2, CH, HW], F8, "k8")                           # 16KB/p
        vt8 = sb([P, 2, NPAIR, F], F8, "vt8")                       # 16KB/p
        w8 = {nm: sb([P, 2, CH * C], F8, f"{nm}8s") for nm in W_ORDER}
        probs8 = [sb([P, 2, F], F8, f"probs{i}") for i in range(8)]
        o8 = [sb([P, 2, F], F8, f"o8_{ch}") for ch in range(CH)]    # 2KB/p
        ot = [sb([P, F], F32, f"ot{i}") for i in range(4)]          # 8KB/p
        rb_sb = sb([P, F], F32, "rb_sb")
        ones8 = sb([P, 2, P], F8, "ones8")
        nshift = sb([P, 1], F32, "nshift")
        misc_sb = sb([P, 16], F32, "misc_sb")
        bq_sb = [misc_sb[:, k:k + 1] for k in range(NT)]
        bk_sb = [misc_sb[:, 4 + k:5 + k] for k in range(NT)]
        gsc_sb = [misc_sb[:, 8 + k:9 + k] for k in range(NT)]
        gbi_sb = [misc_sb[:, 12 + k:13 + k] for k in range(NT)]
        gmat_sb = sb([P, NGT], F32, "gmat_sb")
        gexp_sb = sb([NGT, P], F32, "gexp_sb")
        # groupnorm scratch
        stats = [sb([P, HW // F, 6], F32, f"stats{i}") for i in range(2)]
        mv = sb([P, 2], F32, "mv")
        st2 = sb([P, 2], F32, "st2")
        g2 = sb([NGT, 2], F32, "g2")
        gv = sb([NGT, 1], F32, "gv")
        eps_sb = sb([NGT, 1], F32, "eps_sb")
        chs = sb([P, 2], F32, "chs")
        av = [sb([P, 1], F32, f"av{k}") for k in range(2)]
        bv_ = [sb([P, 1], F32, f"bv{k}") for k in range(2)]

        # ---------------- PSUM: one tensor, all 8 banks ----------------
        # attention: banks 0-2 scores/proj rotation, 3-6 o accum, 7 aux/sums
        # qkv phase: quad staging sets = banks 0-3 / 4-7
        PS = ctx.enter_context(nc.psum_tensor("PS", [P, 8, F], F32))

        # ---------------- semaphores ----------------
        def sem(name):
            return ctx.enter_context(nc.semaphore(name))
        dma_x = [sem(f"dma_x{k}") for k in range(NT)]   # +16 per x tile
        dma_w = sem("dma_w")        # +16 per weight load
        dma_m = sem("dma_m")        # +16 per misc const load
        dma_o = [sem(f"dma_o{i}") for i in range(4)]  # +16/store, by ot slot
        s_dve = sem("s_dve")        # serialized gn DVE chain + memsets
        s_gn_pe = sem("s_gn_pe")    # gn PE matmuls (2 per c-tile)
        s_gn_act = sem("s_gn_act")  # gn sqrt (1 per c-tile)
        s_h = sem("s_h")            # h8 tiles done (ACT)
        s_qg_pe = sem("s_qg_pe")    # qkv matmul groups done (PE)
        s_dr = [sem(f"s_dr{e}") for e in range(3)]  # drains per engine
        s_sc = sem("s_sc")          # scores groups (PE)
        s_exp = sem("s_exp")        # exps (ACT)
        s_att = sem("s_att")        # attnV pairs done (PE)
        s_rb = sem("s_rb")          # rb_sb copy per quarter (DVE)
        s_o8 = sem("s_o8")          # o8 drains (DVE)
        s_pp = sem("s_pp")          # proj matmul groups (PE)
        s_ot = sem("s_ot")          # ot copies (Pool)

        NMISC = 3                   # gmat, gexp, misc16
        marks = {}                  # GN s_dve counts by label
        NMS = 3                     # DVE memsets before the GN chain
        GTL = 21                    # DVE ops per GN tile

        # consumer of banks-0-2 use u (for slot reuse by use u+3)
        def use_consumer(u):
            kind, a, b = EMISSION[u]
            if kind == "score":
                return (s_exp, 32 * a + b + 1)
            return (s_ot, 4 * a + b + 1)

        with nc.Block() as block:

            # ============ SP: all input DMA ============
            @block.sync
            def _(s):
                s.dma_start(out=x_sb[0][:, :],
                            in_=x_d[0:P, :]).then_inc(dma_x[0], 16)
                s.dma_start(out=gmat_sb[:, :], in_=gmat_d[:, :]).then_inc(
                    dma_m, 16)
                s.dma_start(out=gexp_sb[:, :], in_=gexp_d[:, :]).then_inc(
                    dma_m, 16)
                s.dma_start(out=misc_sb[:, :], in_=misc_d[:, :]).then_inc(
                    dma_m, 16)
                for k in range(1, NT):
                    s.dma_start(out=x_sb[k][:, :],
                                in_=x_d[k * P:(k + 1) * P, :]).then_inc(
                        dma_x[k], 16)
                for nm in W_ORDER:
                    s.dma_start(out=w8[nm][:, :, :],
                                in_=w_d[nm][:, :, :]).then_inc(dma_w, 16)

            # ============ DVE ============
            @block.vector
            def _(v):
                waited = {}

                def vwait(semo, val):
                    if val > waited.get(id(semo), -1):
                        v.wait_ge(semo, val)
                        waited[id(semo)] = val

                nc.vector.memset(ones8[:, :, :], 1.0).then_inc(s_dve, 1)
                nc.vector.memset(nshift[:, :], -SHIFT).then_inc(s_dve, 1)
                nc.vector.memset(eps_sb[:, :], EPS).then_inc(s_dve, 1)
                vwait(dma_m, 16 * NMISC)
                ndve = NMS

                def step(op):
                    nonlocal ndve
                    op.then_inc(s_dve, 1)
                    ndve += 1

                # --- groupnorm: per-tile bn_stats, then a split chain
                # A (aggr..var) / B (rstd..a,b) so the ACT sqrt and PE
                # group-combine round trips overlap the next tile's stats ---
                def gn_stats(k):
                    vwait(dma_x[k], 16)
                    for c8 in range(HW // F):
                        step(nc.vector.bn_stats(
                            out=stats[k % 2][:, c8, :],
                            in_=x_sb[k][:, c8 * F:(c8 + 1) * F]))

                def gn_a(k):
                    vwait(s_dve, ndve)
                    if k >= 1:
                        vwait(s_gn_pe, 2 * k - 1)   # st2 read by gmat(k-1)
                    step(nc.vector.bn_aggr(out=mv[:, :],
                                           in_=stats[k % 2][:, :, :]))
                    vwait(s_dve, ndve)
                    step(nc.vector.tensor_copy(out=st2[:, 0:1],
                                               in_=mv[:, 0:1]))
                    step(nc.vector.tensor_mul(out=st2[:, 1:2], in0=mv[:, 0:1],
                                              in1=mv[:, 0:1]))
                    vwait(s_dve, ndve)
                    step(nc.vector.tensor_add(out=st2[:, 1:2],
                                              in0=st2[:, 1:2],
                                              in1=mv[:, 1:2]))
                    marks["st2", k] = ndve
                    vwait(s_gn_pe, 2 * k + 1)           # group sums in aux
                    step(nc.vector.tensor_scalar_mul(g2[:, :],
                                                     in0=PS[0:NGT, 7, 0:2],
                                                     scalar1=1.0 / GS))
                    vwait(s_dve, ndve)
                    step(nc.vector.tensor_mul(out=gv[:, :], in0=g2[:, 0:1],
                                              in1=g2[:, 0:1]))
                    vwait(s_dve, ndve)
                    step(nc.vector.tensor_sub(out=gv[:, :], in0=g2[:, 1:2],
                                              in1=gv[:, :]))
                    marks["gv", k] = ndve

                def gn_b(k):
                    vwait(s_gn_act, k + 1)              # sqrt done
                    step(nc.vector.reciprocal(out=gv[:, :], in_=gv[:, :]))
                    vwait(s_dve, ndve)
                    step(nc.vector.tensor_copy(out=g2[:, 1:2],
                                               in_=gv[:, :]))
                    marks["g2", k] = ndve
                    vwait(s_gn_pe, 2 * k + 2)           # per-channel in aux
                    step(nc.vector.tensor_copy(out=chs[:, :],
                                               in_=PS[0:P, 7, 0:2]))
                    if k >= 2:
                        vwait(s_h, k - 1)       # av/bv_ slot free
                    vwait(s_dve, ndve)
                    step(nc.vector.tensor_mul(out=av[k % 2][:, :],
                                              in0=chs[:, 1:2],
                                              in1=gsc_sb[k]))
                    vwait(s_dve, ndve)
                    step(nc.vector.tensor_mul(out=bv_[k % 2][:, :],
                                              in0=chs[:, 0:1],
                                              in1=av[k % 2][:, :]))
                    vwait(s_dve, ndve)
                    step(nc.vector.tensor_sub(out=bv_[k % 2][:, :],
                                              in0=gbi_sb[k],
                                              in1=bv_[k % 2][:, :]))
                    marks["ab", k] = ndve

                gn_stats(0)
                gn_a(0)
                gn_stats(1)
                gn_b(0)
                gn_a(1)
                gn_stats(2)
                gn_b(1)
                gn_a(2)
                gn_stats(3)
                gn_b(2)
                gn_a(3)
                gn_b(3)
                # --- half the qkv drains (even quads) ---
                for qd, (kind, n) in enumerate(QUADS):
                    if QUAD_ENG[qd] != 0:
                        continue
                    vwait(s_qg_pe, 4 * (qd + 1))
                    base = (qd % 2) * 4
                    if kind == "v":
                        # banks hold j0,j2,j1,j3: target [i, pair, c] matches
                        op = nc.vector.tensor_copy(
                            out=vt8[:, :, 2 * n:2 * n + 2, :],
                            in_=PS[:, base:base + 4, :])
                        op.then_inc(s_dr[0], 1)
                    elif kind == "k":
                        nc.vector.tensor_copy(
                            out=k8[:, :, :, n * F:(n + 1) * F],
                            in_=PS[:, base:base + 4, :]).then_inc(s_dr[0], 1)
                    else:
                        for m in range(4):
                            nc.vector.tensor_scalar_add(
                                out=q8[m // 2][:, m % 2, n * F:(n + 1) * F],
                                in0=PS[:, base + m, :],
                                scalar1=bq_sb[m]).then_inc(s_dr[0], 1)
                # --- attention epilogue per quarter ---
                for qq in range(NQF):
                    vwait(s_att, 5 * NPAIR * (qq + 1))
                    if qq > 0:
                        vwait(s_o8, 4 * qq)     # rb read by prior o8 drains
                    nc.vector.reciprocal(
                        out=rb_sb[:, :],
                        in_=PS[:, 7, :]).then_inc(s_rb, 1)
                    vwait(s_pp, 4 * qq)         # o8 read by prior proj
                    vwait(s_rb, qq + 1)         # rb write complete (own pipe)
                    for c4 in range(NT):
                        nc.vector.tensor_mul(
                            out=o8[c4 // 2][:, c4 % 2, :],
                            in0=PS[:, 3 + c4, :],
                            in1=rb_sb[:, :]).then_inc(s_o8, 1)
                    # proj psum -> ot staging (stores issued by SP)
                    for o4 in range(NT):
                        st = 4 * qq + o4
                        vwait(s_pp, st + 1)
                        if st >= 4:
                            vwait(dma_o[st % 4], 16 * (st // 4))
                        u = PROJ_U[(qq, o4)]
                        nc.vector.tensor_copy(
                            out=ot[st % 4][:, :],
                            in_=PS[:, u % 3, :]).then_inc(s_ot, 1)

            # ============ PE: all matmuls ============
            @block.tensor
            def _(t):
                waited = {}

                def twait(semo, val):
                    if val > waited.get(id(semo), -1):
                        t.wait_ge(semo, val)
                        waited[id(semo)] = val

                # --- groupnorm group-combine matmuls ---
                twait(dma_m, 16 * NMISC)
                for k in range(NT):
                    twait(s_dve, marks["st2", k])       # st2 ready
                    nc.tensor.matmul(PS[0:NGT, 7, 0:2], lhsT=gmat_sb[:, :],
                                     rhs=st2[:, :], start=True,
                                     stop=True).then_inc(s_gn_pe, 1)
                    twait(s_dve, marks["g2", k])        # g2 ready
                    nc.tensor.matmul(PS[0:P, 7, 0:2], lhsT=gexp_sb[:, :],
                                     rhs=g2[:, :], start=True,
                                     stop=True).then_inc(s_gn_pe, 1)
                # --- qkv fp8 DR matmuls staged in bank quads ---
                twait(dma_w, 16 * 4)
                twait(s_h, NT)
                for qd, (kind, n) in enumerate(QUADS):
                    if qd >= 2:
                        e, cnt = QUAD_DRAIN[qd - 2]
                        twait(s_dr[e], cnt)
                    base = (qd % 2) * 4
                    for gi in range(4):
                        dst = PS[:, base + gi, :]
                        for ch in range(CH):
                            kw = dict(start=(ch == 0), stop=(ch == CH - 1),
                                      perf_mode=DR)
                            if kind == "v":
                                # bank order j0,j2,j1,j3 so the quad drain's
                                # (i, pair) target iteration matches banks
                                j = 4 * n + (gi % 2) * 2 + gi // 2
                                mm = nc.tensor.matmul(
                                    dst, lhsT=h8[ch][:, :, j * P:(j + 1) * P],
                                    rhs=w8["wv"][:, :, ch * C:(ch + 1) * C],
                                    **kw)
                            else:
                                # K: bank order (i, ch) = m0,m2,m1,m3 so the
                                # single-instr drain target iteration matches
                                m = (gi % 2) * 2 + gi // 2 if kind == "k" else gi
                                wname = "wq" if kind == "q" else "wk"
                                mm = nc.tensor.matmul(
                                    dst,
                                    lhsT=w8[wname][:, :, ch * C + m * P:
                                                   ch * C + (m + 1) * P],
                                    rhs=h8[ch][:, :, n * F:(n + 1) * F],
                                    **kw)
                        mm.then_inc(s_qg_pe, 1)

                # --- attention + proj (banks 0-2 rotation starts at u=0) ---
                def slot_wait(u):
                    if u >= 3:
                        semo, val = use_consumer(u - 3)
                        twait(semo, val)
                    else:
                        e, cnt = QUAD_DRAIN[LAST_QUAD_IN_BANK[u % 3]]
                        twait(s_dr[e], cnt)

                def score(qq, j):
                    u = SCORE_U[(qq, j)]
                    slot_wait(u)
                    for e, cnt in _drain_waits(Q_DONE[qq] + K_DONE[j // 4]):
                        twait(s_dr[e], cnt)
                    for ch in range(CH):
                        mm = nc.tensor.matmul(
                            PS[:, u % 3, :],
                            lhsT=k8[:, :, ch, j * P:(j + 1) * P],
                            rhs=q8[ch][:, :, qq * F:(qq + 1) * F],
                            start=(ch == 0), stop=(ch == CH - 1),
                            perf_mode=DR)
                    mm.then_inc(s_sc, 1)

                def attnv(qq, b):
                    # pair b: j = 2b, 2b+1 in probs8[(16qq+b) % 8]
                    twait(s_exp, 32 * qq + 2 * b + 2)
                    for e, cnt in _drain_waits(V_DONE[b]):
                        twait(s_dr[e], cnt)
                    if b == 0:
                        twait(s_o8, 4 * qq)     # o_ps read by prior o8 drain
                        twait(s_rb, qq)         # aux read by prior rb copy
                        if qq == 0:
                            for bank in (3, 4, 5, 6, 7):
                                e, cnt = QUAD_DRAIN[LAST_QUAD_IN_BANK[bank]]
                                twait(s_dr[e], cnt)
                    kw = dict(start=(b == 0), stop=(b == NPAIR - 1))
                    # ones lhsT [P,2,128]: every out partition = key-sums,
                    # i.e. the softmax denominators pre-broadcast
                    nc.tensor.matmul(PS[:, 7, :], lhsT=ones8[:, :, :],
                                     rhs=probs8[(16 * qq + b) % 8][:, :, :],
                                     perf_mode=DR, **kw).then_inc(s_att, 1)
                    for c4 in range(NT):
                        nc.tensor.matmul(
                            PS[:, 3 + c4, :],
                            lhsT=vt8[:, :, b, c4 * P:(c4 + 1) * P],
                            rhs=probs8[(16 * qq + b) % 8][:, :, :],
                            perf_mode=DR, **kw).then_inc(s_att, 1)

                for qq in range(NQF):
                    if qq == 0:
                        for j in range(LOOKAHEAD):
                            score(0, j)
                    for b in range(LOOKAHEAD // 2 - 1):
                        attnv(qq, b)
                    for b in range(LOOKAHEAD // 2 - 1, NPAIR - 1):
                        score(qq, 2 * b + 2)
                        score(qq, 2 * b + 3)
                        attnv(qq, b)
                    attnv(qq, NPAIR - 1)
                    # epilogue: lookahead scores keep ACT fed while the
                    # recip/o8 chain runs on DVE
                    if qq + 1 < NQF:
                        for j in range(LOOKAHEAD):
                            score(qq + 1, j)
                    twait(s_o8, 4 * (qq + 1))
                    for o4 in range(NT):
                        u = PROJ_U[(qq, o4)]
                        slot_wait(u)
                        for ch in range(CH):
                            mm = nc.tensor.matmul(
                                PS[:, u % 3, :],
                                lhsT=w8["wp"][:, :, ch * C + o4 * P:
                                              ch * C + (o4 + 1) * P],
                                rhs=o8[ch][:, :, :],
                                start=(ch == 0), stop=(ch == CH - 1),
                                perf_mode=DR)
                        mm.then_inc(s_pp, 1)

            # ============ ACT: sqrt, h8 writes, drains, exps ============
            @block.scalar
            def _(a):
                waited = {}

                def awaitg(semo, val):
                    if val > waited.get(id(semo), -1):
                        a.wait_ge(semo, val)
                        waited[id(semo)] = val

                def gn_sqrt(k):
                    awaitg(s_dve, marks["gv", k])
                    nc.scalar.activation(
                        out=gv[:, :], in_=gv[:, :], func=AF.Sqrt,
                        bias=eps_sb[:, :]).then_inc(s_gn_act, 1)

                def gn_h(k):
                    awaitg(s_dve, marks["ab", k])
                    nc.scalar.activation(
                        out=h8[k // 2][:, k % 2, :], in_=x_sb[k][:, :],
                        func=AF.Identity, bias=bv_[k % 2][:, :],
                        scale=av[k % 2][:, :]).then_inc(s_h, 1)

                gn_sqrt(0)
                gn_sqrt(1)
                gn_h(0)
                gn_sqrt(2)
                gn_h(1)
                gn_sqrt(3)
                gn_h(2)
                gn_h(3)
                # --- half the qkv drains (odd quads) ---
                for qd, (kind, n) in enumerate(QUADS):
                    if QUAD_ENG[qd] != 1:
                        continue
                    awaitg(s_qg_pe, 4 * (qd + 1))
                    base = (qd % 2) * 4
                    if kind == "v":
                        op = nc.scalar.activation(
                            out=vt8[:, :, 2 * n:2 * n + 2, :],
                            in_=PS[:, base:base + 4, :], func=AF.Copy)
                        op.then_inc(s_dr[1], 1)
                    elif kind == "k":
                        nc.scalar.activation(
                            out=k8[:, :, :, n * F:(n + 1) * F],
                            in_=PS[:, base:base + 4, :],
                            func=AF.Copy).then_inc(s_dr[1], 1)
                    else:
                        for m in range(4):
                            nc.scalar.activation(
                                out=q8[m // 2][:, m % 2, n * F:(n + 1) * F],
                                in_=PS[:, base + m, :], func=AF.Identity,
                                bias=bq_sb[m]).then_inc(s_dr[1], 1)
                # --- exps ---
                for qq in range(NQF):
                    for j in range(NJ):
                        e = 32 * qq + j
                        pa = 16 * qq + j // 2       # absolute pair index
                        awaitg(s_sc, e + 1)
                        if pa >= 8:
                            awaitg(s_att, 5 * (pa - 7))  # probs buffer free
                        u = SCORE_U[(qq, j)]
                        nc.scalar.activation(
                            out=probs8[(16 * qq + j // 2) % 8][:, j % 2, :],
                            in_=PS[:, u % 3, :],
                            func=AF.Exp, bias=nshift[:, :],
                            scale=SCALE).then_inc(s_exp, 1)

            # ============ Pool: drains + ot copies + output stores ============
            @block.gpsimd
            def _(gp):
                waited = {}

                def gwait(semo, val):
                    if val > waited.get(id(semo), -1):
                        gp.wait_ge(semo, val)
                        waited[id(semo)] = val

                for g, grp in enumerate(QKV_GROUPS):
                    if g % 3 != 2:
                        continue
                    gwait(s_qg_pe, g + 1)
                    src = sc_ps[:, g % 3, :]
                    if grp[0] == "v":
                        j = grp[1]
                        op = nc.gpsimd.tensor_copy(
                            out=vt8[:, j // 2, j % 2, :], in_=src)
                    elif grp[0] == "q":
                        _, m, n = grp
                        op = nc.gpsimd.tensor_scalar_add(
                            out=q8[:, m // 2, m % 2, n * F:(n + 1) * F],
                            in0=src, scalar1=bq_sb[m])
                    else:
                        _, m, n = grp
                        op = nc.gpsimd.tensor_scalar_add(
                            out=k8[:, m // 2, m % 2, n * F:(n + 1) * F],
                            in0=src, scalar1=bk_sb[m])
                    op.then_inc(s_dr[2], 1)
                # --- ot copies + output stores ---
                for qq in range(NQF):
                    for o4 in range(NT):
                        st = 4 * qq + o4
                        gwait(s_pp, st + 1)
                        if st >= 2:
                            # ot slot free when the slot's previous store done
                            gwait(dma_o[st % 2], 16 * (st // 2))
                        u = PROJ_U[(qq, o4)]
                        nc.gpsimd.tensor_copy(
                            out=ot[st % 2][:, :],
                            in_=sc_ps[:, u % 3, :]).then_inc(s_ot, 1)
                        gwait(s_ot, st + 1)     # ot write complete (own pipe)
                        gp.dma_start(
                            out=out_d[o4 * P:(o4 + 1) * P,
                                      qq * F:(qq + 1) * F],
                            in_=ot[st % 4][:, :]).then_inc(dma_o[st % 4], 16)

    return nc


def _f8(a):
    return np.ascontiguousarray(np.asarray(a, ml_dtypes.float8_e4m3))


def _pair_layout(wT):
    # wT [c_in, c_out] f32 -> [p, i, (ch, c_out)] fp8,
    # c_in = ch*256 + i*128 + p
    return _f8(wT.reshape(CH, 2, P, C).transpose(2, 1, 0, 3).reshape(
        P, 2, CH * C))


def make_in_maps(x, gn_scale, gn_bias, qkv_w, qkv_b, proj_w, proj_b):
    xf = np.ascontiguousarray(x, dtype=np.float32).reshape(B, C, HW)
    qkv_w = np.asarray(qkv_w, np.float32)
    wq, wk, wv = qkv_w[0:C], qkv_w[C:2 * C], qkv_w[2 * C:3 * C]
    wts = {"wq": wq.T, "wk": wk.T, "wv": wv.T,
           "wp": np.asarray(proj_w, np.float32).T}
    misc16 = np.stack(
        [np.asarray(qkv_b[0:C], np.float32).reshape(NT, P),
         np.asarray(qkv_b[C:2 * C], np.float32).reshape(NT, P),
         np.asarray(gn_scale, np.float32).reshape(NT, P),
         np.asarray(gn_bias, np.float32).reshape(NT, P)],
    ).reshape(16, P).T          # [P, 16]: bq0..3 | bk0..3 | gsc0..3 | gbi0..3
    shared = {
        **{f"{nm}8": _pair_layout(np.ascontiguousarray(w, np.float32))
           for nm, w in wts.items()},
        "misc16": np.ascontiguousarray(misc16),
        "gmat": np.ascontiguousarray(
            (np.arange(P)[:, None] // GS == np.arange(NGT)[None, :]),
            np.float32),
        "gexp": np.ascontiguousarray(
            (np.arange(NGT)[:, None] == np.arange(P)[None, :] // GS),
            np.float32),
    }
    in_maps = []
    for b in range(B):
        for half in range(2):
            xr = np.ascontiguousarray(np.roll(xf[b], -half * NQ, axis=1))
            in_maps.append({"x": xr, **shared})
    return in_maps, xf


def assemble(results, xf, qkv_b, proj_w, proj_b):
    out = np.empty((B, C, HW), np.float32)
    i = 0
    for b in range(B):
        for half in range(2):
            out[b][:, half * NQ:(half + 1) * NQ] = results[i]["out"]
            i += 1
    # device skips the V bias (attn rows sum to 1, so it contributes wp@bv)
    # and the proj bias; both are folded here along with the residual
    extra = (np.asarray(proj_w, np.float32)
             @ np.asarray(qkv_b[2 * C:3 * C], np.float32)
             + np.asarray(proj_b, np.float32))
    out += extra[None, :, None]
    out += xf
    return out.reshape(B, C, H, W)


def kernel(x, gn_scale, gn_bias, qkv_w, qkv_b, proj_w, proj_b):
    in_maps, xf = make_in_maps(x, gn_scale, gn_bias, qkv_w, qkv_b,
                               proj_w, proj_b)
    nc = build_nc()
    res = run_bass_kernel_spmd(nc, in_maps, list(range(8)))
    return assemble(res.results, xf, qkv_b, proj_w, proj_b)
